# revision 9
# baseline (speedup 1.0000x reference)
# ContextRCNN attention-bias kernel for 8 Trainium2 NeuronCores — bf16 build.
#
# Reference computation:
#   central:[N,C,7,7] -> mean-pool -> Q-MLP -> l2norm -> queries [N,QK]
#   context:[T,C,7,7] -> mean-pool -> K/V-MLPs (K l2normed)
#   softmax(Q @ K^T * 6.25) @ V -> final MLP -> [N, C]
#
# Distribution (one SPMD NEFF on 8 cores, rank == q-shard == t-shard index):
#   central rows sharded N/8=512 per core; context rows sharded T/8=1536.
#   Local Q -> AllGather; every core attends ALL 4096 queries against its
#   local K/V slab producing partial [4096, 258] numerators (V plus a ones
#   column for the softmax denominator); ReduceScatter sums partials and
#   hands rank r its 512 rows; divide + final MLP locally.
#
# Speed levers vs the fp32 build (606us):
#   * Inputs + weights cast to bf16 on host: HBM traffic halves to ~52MB/core
#     and every matmul runs at 1 PE cycle/row instead of 4.
#   * Mean-pool via a binary tree of 2-byte tensor_adds (DVE runs packed
#     16-bit elementwise at 2x; InstTensorReduce has no fast mode). Big tree
#     levels on DVE; tail levels for the context stream on GpSimd.
#   * Chunk-level software pipeline: engine queues are strict FIFO, so the
#     next chunk's pooling is emitted interleaved with the current chunk's
#     attention, and the K/V MLP between chunks — chunk-0 attention stalling
#     on the query AllGather no longer blocks later pooling/MLP work.
#   * K rows are NOT normalized: 1/||k|| is folded into exp's per-partition
#     scale operand. 1/||x|| is computed as exp(-0.5*ln(ssq)) — ln and exp
#     live in the same ACT table set, so no act-table thrash (sqrt doesn't).
#   * Numerators accumulate in fp32 PSUM within a chunk; cross-chunk adds on
#     DVE; rs_in rows DMA out inline as their last add retires.
# Host emulation of this precision mix gives rel err ~5e-3 (gate is 2e-2).

import numpy as np
from contextlib import ExitStack

import ml_dtypes
import concourse.bass as bass
import concourse.mybir as mybir
import concourse.tile as tile
from concourse import bacc
from concourse.bass_utils import run_bass_kernel_spmd
from concourse.masks import make_identity

AF = mybir.ActivationFunctionType
F32 = mybir.dt.float32
BF = mybir.dt.bfloat16
F16 = mybir.dt.float16

M = 8                    # cores
N, T, C, S = 4096, 12288, 256, 7
NS, TS = N // M, T // M  # 512 q rows / 1536 kv rows per core
H = 512                  # MLP hidden
D = 256                  # QK == VD == C
SS = S * S               # 49
SCALE = 1.0 / (0.01 * C ** 0.5)   # 6.25
NT_Q = NS // 128         # 4  q-tiles per core
NT_T = TS // 128         # 12 t-tiles per core
NCOL = D + 2             # V plus ones cols
HALF = 128 * SS          # elems per half row-tile line (128 channels x 49)
CH_LIST = [3, 4, 5]      # t-tiles per context chunk (small first chunk so
NCHUNK = len(CH_LIST)    # attention starts early; 3+4+5 = 12)
CH0 = [sum(CH_LIST[:i]) for i in range(NCHUNK)]   # first t-tile of chunk
HIDW = max(max(CH_LIST) * 128, NS)   # fixed hidden-tile width (tag ring reuse)


def build_nc():
    nc = bacc.Bacc("TRN2", target_bir_lowering=False, debug=False, num_devices=M)

    central = nc.dram_tensor("central_sh", [NS, C, S, S], BF, kind="ExternalInput")
    context = nc.dram_tensor("context_sh", [TS, C, S, S], BF, kind="ExternalInput")
    wnames = ["qw1", "qw2", "kw1", "kw2", "vw1", "vw2", "fw1", "fw2"]
    wshapes = {"1": [C, H], "2": [H, D]}
    wdram = {n: nc.dram_tensor(n, wshapes[n[-1]], BF, kind="ExternalInput")
             for n in wnames}
    out_sh = nc.dram_tensor("out_sh", [NS, C], F32, kind="ExternalOutput")

    qt_in = nc.dram_tensor("qt_in", [D, NS], BF)                       # AG input
    qt_out = nc.dram_tensor("qt_out", [M * D, NS], BF, addr_space="Shared")
    rs_in = nc.dram_tensor("rs_in", [N, NCOL], F32)                    # RS input
    rs_out = nc.dram_tensor("rs_out", [NS, NCOL], F32)

    with tile.TileContext(nc) as tc, ExitStack() as ctx:
        ident_pool = ctx.enter_context(tc.tile_pool(name="ident", bufs=1))
        ident = ident_pool.tile([128, 128], BF)
        make_identity(nc, ident[:])

        # SBUF tensors that live across phases
        kvq_pool = ctx.enter_context(tc.tile_pool(name="kvq", bufs=1))
        qt_all = [kvq_pool.tile([128, NS], BF, tag=f"qta{i}", name=f"qta{i}")
                  for i in range(2 * M)]
        krcp = [kvq_pool.tile([128, 1], F32, tag=f"krcp{j}", name=f"krcp{j}")
                for j in range(NT_T)]
        # numerator accumulators for all 32 q-tiles (summed over chunks)
        nm_pool = ctx.enter_context(tc.tile_pool(name="nm", bufs=1))
        nm_sb = [nm_pool.tile([128, NCOL], F32, tag=f"nm{i}", name=f"nm{i}")
                 for i in range(N // 128)]

        def transpose128(dst_sb, src_sb, tp_pool, eng="vector"):
            """dst[128,128](bf16) = src[128,128]^T via PE (copy-out on eng)."""
            ps = tp_pool.tile([128, 256], BF, tag="ps_small", name="tp")
            nc.tensor.transpose(ps[:, 0:128], src_sb, ident[:])
            if eng == "scalar":
                nc.scalar.copy(dst_sb, ps[:, 0:128])
            else:
                nc.vector.tensor_copy(dst_sb, ps[:, 0:128])

        def load_w(pool, name):
            shape = wshapes[name[-1]]
            tiles = []
            for i in range(shape[0] // 128):
                t = pool.tile([128, shape[1]], BF, tag=f"{name}_{i}",
                              name=f"{name}_{i}")
                nc.sync.dma_start(t[:], wdram[name].ap()[i * 128:(i + 1) * 128, :])
                tiles.append(t)
            return tiles

        def pool_rows(dram_t, row0, raw_pool, sA_pool, sB_pool, pooled_pool,
                      tail_eng="gpsimd"):
            """Sum-pool 128 rows of [rows,C,7,7](bf16) -> pooled [128, C] bf16.
            Binary tree of packed 16-bit adds: 49 = (((24+24)->12->6->3)->1)+1.
            Levels 1-3 on DVE, tail on tail_eng. (The 1/49 mean scale cancels
            in l2norm for Q/K and is folded into vw2 on the host for V.)"""
            pooled = pooled_pool.tile([128, C], BF)
            src = dram_t.ap().rearrange("t c h w -> t (c h w)")
            tail = nc.gpsimd if tail_eng == "gpsimd" else nc.vector
            for h in range(2):
                raw = raw_pool.tile([128, HALF], BF, tag="raw")
                nc.sync.dma_start(
                    raw[:], src[row0:row0 + 128, h * HALF:(h + 1) * HALF])
                a = raw[:].rearrange("p (c s) -> p c s", s=SS)
                sA_t = sA_pool.tile([128, 128 * 42], F16, tag="sA", name="sA")
                sA = sA_t[:].rearrange("p (c s) -> p c s", s=42)
                s1, s2, s3 = sA[:, :, 0:24], sA[:, :, 24:36], sA[:, :, 36:42]
                nc.vector.tensor_add(s1, a[:, :, 0:24], a[:, :, 24:48])
                nc.vector.tensor_add(s2, s1[:, :, 0:12], s1[:, :, 12:24])
                nc.vector.tensor_add(s3, s2[:, :, 0:6], s2[:, :, 6:12])
                sB_t = sB_pool.tile([128, 128 * 5], F16, tag="sB", name="sB")
                sB = sB_t[:].rearrange("p (c s) -> p c s", s=5)
                s4, s5, s6 = sB[:, :, 0:3], sB[:, :, 3:4], sB[:, :, 4:5]
                tail.tensor_add(s4, s3[:, :, 0:3], s3[:, :, 3:6])
                tail.tensor_add(s5, s4[:, :, 0:1], s4[:, :, 1:2])
                tail.tensor_add(s6, s5, s4[:, :, 2:3])
                po = pooled[:, h * 128:(h + 1) * 128] \
                    .rearrange("p (c s) -> p c s", s=1)
                tail.tensor_add(po, s6, a[:, :, 48:49])
            return pooled

        def mlp_l1_T(w1_tiles, xT, out_tiles, nfree, ps_pool):
            """hidden^T[h,n](bf16) = relu(w1^T @ x^T) over <=512-wide blocks."""
            for ht in range(H // 128):
                for c0 in range(0, nfree, 512):
                    w = min(512, nfree - c0)
                    ps = ps_pool.tile([128, 512], F32, tag="mm512")
                    for ck in range(C // 128):
                        nc.tensor.matmul(
                            ps[:, 0:w],
                            w1_tiles[ck][:, ht * 128:(ht + 1) * 128],
                            xT[ck][:, c0:c0 + w],
                            start=(ck == 0), stop=(ck == 1))
                    nc.scalar.activation(
                        out_tiles[ht][:, c0:c0 + w], ps[:, 0:w], AF.Relu)

        def mlp_l2_nat(hid_tiles, w2_tiles, nt, ps_pool):
            """x[n,d] psum tile = hidden @ w2 for 128-row block nt."""
            ps = ps_pool.tile([128, 256], F32, tag="ps_small", name="l2ps")
            for hk in range(H // 128):
                nc.tensor.matmul(
                    ps[:],
                    hid_tiles[hk][:, nt * 128:(nt + 1) * 128],
                    w2_tiles[hk][:],
                    start=(hk == 0), stop=(hk == 3))
            return ps

        def norm_recip(src_ps, pool, out=None):
            """1/||row|| as [128,1] f32 from psum tile, via exp(-0.5*ln(ssq)).
            ln+exp share an ACT table set with exp/relu/copy -> no table
            thrash (sqrt lives in a different set)."""
            sq = pool.tile([128, D], F32, tag="sq", name="sq")
            nc.scalar.activation(sq[:], src_ps[:], AF.Square)
            ssq = pool.tile([128, 1], F32, tag="ssq", name="ssq")
            nc.vector.reduce_sum(ssq[:], sq[:], axis=mybir.AxisListType.X)
            lg = pool.tile([128, 1], F32, tag="lg", name="lg")
            nc.scalar.activation(lg[:], ssq[:], AF.Ln)
            if out is None:
                out = pool.tile([128, 1], F32, tag="rcp", name="rcp")
            nc.scalar.activation(out[:], lg[:], AF.Exp, scale=-0.5)
            return out

        with tc.tile_pool(name="raw", bufs=5) as raw_pool, \
             tc.tile_pool(name="sA", bufs=2) as sA_pool, \
             tc.tile_pool(name="sB", bufs=2) as sB_pool, \
             tc.tile_pool(name="pooled", bufs=3) as pooled_pool, \
             tc.tile_pool(name="ptq", bufs=1) as ptq_pool, \
             tc.tile_pool(name="ptc", bufs=2) as ptc_pool, \
             tc.tile_pool(name="wA", bufs=1) as wA_pool, \
             tc.tile_pool(name="hid", bufs=2) as hid_pool, \
             tc.tile_pool(name="kv", bufs=2) as kv_pool, \
             tc.tile_pool(name="small", bufs=2) as small_pool, \
             tc.tile_pool(name="est", bufs=12) as e_pool, \
             tc.tile_pool(name="ps512", bufs=2, space="PSUM") as ps512, \
             tc.tile_pool(name="psSm", bufs=2, space="PSUM") as psSm, \
             tc.tile_pool(name="psST", bufs=2, space="PSUM") as ps_st, \
             tc.tile_pool(name="psNM", bufs=2, space="PSUM") as ps_nm:

            qw1 = load_w(wA_pool, "qw1"); qw2 = load_w(wA_pool, "qw2")
            kw1 = load_w(wA_pool, "kw1"); kw2 = load_w(wA_pool, "kw2")
            vw1 = load_w(wA_pool, "vw1"); vw2 = load_w(wA_pool, "vw2")

            # --- central: pool + transpose -> cenT [c, NS] ---
            cenT = [ptq_pool.tile([128, NS], BF, tag=f"cenT{i}", name=f"cenT{i}")
                    for i in range(2)]
            for ntile in range(NT_Q):
                pooled = pool_rows(central, ntile * 128, raw_pool,
                                   sA_pool, sB_pool, pooled_pool,
                                   tail_eng="vector")
                for ct in range(2):
                    transpose128(cenT[ct][:, ntile * 128:(ntile + 1) * 128],
                                 pooled[:, ct * 128:(ct + 1) * 128], psSm)

            # --- Q MLP, l2norm * 6.25, transpose, AllGather (issued async;
            #     the readback DMAs are emitted later, just before they're
            #     needed, so they don't head-of-line-block the gpsimd queue) ---
            hq = [hid_pool.tile([128, HIDW], BF, tag=f"hid{i}", name=f"hq{i}")
                  for i in range(4)]
            mlp_l1_T(qw1, cenT, hq, NS, ps512)
            qt_sh = [wA_pool.tile([128, NS], BF, tag=f"qtsh{i}", name=f"qtsh{i}")
                     for i in range(2)]
            for ntile in range(NT_Q):
                q_ps = mlp_l2_nat(hq, qw2, ntile, psSm)
                rcp = norm_recip(q_ps, small_pool)
                qn = small_pool.tile([128, D], BF, tag="qn", name="qn")
                nc.vector.tensor_scalar(
                    qn[:], q_ps[:], rcp[:], SCALE,
                    op0=mybir.AluOpType.mult, op1=mybir.AluOpType.mult)
                for ct in range(2):
                    transpose128(qt_sh[ct][:, ntile * 128:(ntile + 1) * 128],
                                 qn[:, ct * 128:(ct + 1) * 128], psSm)
            for ct in range(2):
                nc.gpsimd.dma_start(qt_in.ap()[ct * 128:(ct + 1) * 128, :],
                                    qt_sh[ct][:])
            nc.gpsimd.collective_compute(
                "AllGather", mybir.AluOpType.bypass,
                replica_groups=[list(range(M))],
                ins=[qt_in[:]], outs=[qt_out[:]])

            # --- context chunks: pooling / K/V MLP / attention, chunk-level
            #     software pipeline (pooling of chunk c+1 interleaves with the
            #     attention of chunk c; K/V MLP of c+1 between the two) ---
            ctxT_c = {}   # chunk -> [2] tiles [128, CH*128] bf16
            kt_c = {}     # chunk -> [2] tiles [128, CH*128] bf16 (K^T)
            vo_c = {}     # chunk -> [CH] tiles [128, NCOL] bf16

            CHMAX = max(CH_LIST)

            def pool_one(chunk, j):
                if j == 0:
                    ctxT_c[chunk] = [
                        ptc_pool.tile([128, CHMAX * 128], BF, tag=f"ctxT{i}",
                                      name=f"ctxT{chunk}_{i}") for i in range(2)]
                pooled = pool_rows(context, (CH0[chunk] + j) * 128, raw_pool,
                                   sA_pool, sB_pool, pooled_pool)
                for ct in range(2):
                    transpose128(ctxT_c[chunk][ct][:, j * 128:(j + 1) * 128],
                                 pooled[:, ct * 128:(ct + 1) * 128],
                                 psSm, eng="scalar")

            def kv_mlp(chunk):
                ch = CH_LIST[chunk]
                ctxT = ctxT_c[chunk]
                kt_c[chunk] = [kv_pool.tile([128, CHMAX * 128], BF, tag=f"kt{i}",
                                            name=f"kt{chunk}_{i}")
                               for i in range(2)]
                vo_c[chunk] = [kv_pool.tile([128, NCOL], BF, tag=f"vo{j}",
                                            name=f"vo{chunk}_{j}")
                               for j in range(ch)]
                hk = [hid_pool.tile([128, HIDW], BF, tag=f"hid{i}",
                                    name=f"hk{chunk}_{i}") for i in range(4)]
                mlp_l1_T(kw1, ctxT, hk, ch * 128, ps512)
                for j in range(ch):
                    k_ps = mlp_l2_nat(hk, kw2, j, psSm)
                    norm_recip(k_ps, small_pool, out=krcp[CH0[chunk] + j])
                    kn = small_pool.tile([128, D], BF, tag="qn", name="kn")
                    nc.vector.tensor_copy(kn[:], k_ps[:])
                    for ct in range(2):
                        transpose128(kt_c[chunk][ct][:, j * 128:(j + 1) * 128],
                                     kn[:, ct * 128:(ct + 1) * 128],
                                     psSm, eng="scalar")
                hv = [hid_pool.tile([128, HIDW], BF, tag=f"hid{i}",
                                    name=f"hv{chunk}_{i}") for i in range(4)]
                mlp_l1_T(vw1, ctxT, hv, ch * 128, ps512)
                for j in range(ch):
                    v_ps = mlp_l2_nat(hv, vw2, j, psSm)
                    nc.scalar.copy(vo_c[chunk][j][:, 0:D], v_ps[:])
                    nc.gpsimd.memset(vo_c[chunk][j][:, D:NCOL], 1.0)

            def attn(chunk):
                """Attention of ALL q-blocks vs this chunk's K/V. The next
                chunk's pooling is interleaved per q-block. Emit q-block qb's
                scores+exp before q-block qb-1's numer matmuls so exp latency
                hides under PE work."""
                ch = CH_LIST[chunk]
                kt = kt_c[chunk]
                prev_e = None

                def numer_for(e_tiles, qb):
                    for qt in range(NT_Q):
                        nm_ps = ps_nm.tile([128, NCOL], F32, tag="nmps",
                                           name="nmps")
                        for j in range(ch):
                            nc.tensor.matmul(
                                nm_ps[:],
                                e_tiles[j][:, qt * 128:(qt + 1) * 128],
                                vo_c[chunk][j][:],
                                start=(j == 0), stop=(j == ch - 1),
                                skip_group_check=True)
                        qrow = qb * NT_Q + qt
                        if chunk == 0:
                            nc.scalar.copy(nm_sb[qrow][:], nm_ps[:])
                        else:
                            nc.vector.tensor_add(nm_sb[qrow][:],
                                                 nm_sb[qrow][:], nm_ps[:])
                        if chunk == NCHUNK - 1:
                            nc.sync.dma_start(
                                rs_in.ap()[qrow * 128:(qrow + 1) * 128, :],
                                nm_sb[qrow][:])

                for qb in range(M):
                    e_tiles = []
                    for j in range(ch):
                        st = ps_st.tile([128, 512], F32, tag="st")
                        for dk in range(2):
                            nc.tensor.matmul(
                                st[:], kt[dk][:, j * 128:(j + 1) * 128],
                                qt_all[2 * qb + dk][:],
                                start=(dk == 0), stop=(dk == 1))
                        e_sb = e_pool.tile([128, 512], BF, tag="e", name="esb")
                        nc.scalar.activation(e_sb[:], st[:], AF.Exp,
                                             scale=krcp[CH0[chunk] + j][:])
                        e_tiles.append(e_sb)
                    if prev_e is not None:
                        numer_for(prev_e, qb - 1)
                    prev_e = e_tiles
                    # interleave next chunk's pooling with this chunk's
                    # attention so DVE/GpSimd/DMA stay fed while PE works
                    if chunk + 1 < NCHUNK and qb < CH_LIST[chunk + 1]:
                        pool_one(chunk + 1, qb)
                numer_for(prev_e, M - 1)

            pool_one(0, 0); pool_one(0, 1); pool_one(0, 2)
            kv_mlp(0)
            # AllGather readback — emitted late so the gpsimd queue's earlier
            # work (pool tails) isn't stuck behind the AG wait
            for i in range(2 * M):
                nc.gpsimd.dma_start(qt_all[i][:],
                                    qt_out.ap()[i * 128:(i + 1) * 128, :])
            for chunk in range(NCHUNK):
                attn(chunk)
                if chunk + 1 < NCHUNK:
                    kv_mlp(chunk + 1)

        nc.gpsimd.collective_compute(
            "ReduceScatter", mybir.AluOpType.add,
            replica_groups=[list(range(M))],
            ins=[rs_in[:]], outs=[rs_out[:]])

        # ---------------- divide + final MLP ----------------
        with tc.tile_pool(name="fin", bufs=2) as fin_pool, \
             tc.tile_pool(name="wC", bufs=1) as wC_pool, \
             tc.tile_pool(name="hidC", bufs=1) as hidC_pool, \
             tc.tile_pool(name="psC", bufs=2, space="PSUM") as psC, \
             tc.tile_pool(name="psTC", bufs=2, space="PSUM") as psTC:
            fw1 = load_w(wC_pool, "fw1"); fw2 = load_w(wC_pool, "fw2")
            attnT = [fin_pool.tile([128, NS], BF, tag=f"attnT{i}",
                                   name=f"attnT{i}") for i in range(2)]
            for ntile in range(NT_Q):
                att = fin_pool.tile([128, NCOL], F32, tag="att", name="att")
                nc.sync.dma_start(
                    att[:], rs_out.ap()[ntile * 128:(ntile + 1) * 128, :])
                rd = fin_pool.tile([128, 1], F32, tag="rd", name="rd")
                nc.vector.reciprocal(rd[:], att[:, D:D + 1])
                an = fin_pool.tile([128, D], BF, tag="an", name="an")
                nc.vector.tensor_scalar_mul(an[:], att[:, 0:D], rd[:])
                for ct in range(2):
                    transpose128(attnT[ct][:, ntile * 128:(ntile + 1) * 128],
                                 an[:, ct * 128:(ct + 1) * 128], psTC)
            hf = [hidC_pool.tile([128, NS], BF, tag=f"hidC{i}", name=f"hfC{i}")
                  for i in range(4)]
            mlp_l1_T(fw1, attnT, hf, NS, psC)
            for ntile in range(NT_Q):
                o_ps = psC.tile([128, 256], F32, tag="ops", name="ops")
                for hk_i in range(H // 128):
                    nc.tensor.matmul(
                        o_ps[:], hf[hk_i][:, ntile * 128:(ntile + 1) * 128],
                        fw2[hk_i][:], start=(hk_i == 0), stop=(hk_i == 3))
                o_sb = fin_pool.tile([128, D], F32, tag="osb", name="osb")
                nc.vector.tensor_copy(o_sb[:], o_ps[:])
                nc.sync.dma_start(
                    out_sh.ap()[ntile * 128:(ntile + 1) * 128, :], o_sb[:])

    nc.finalize()
    return nc


_NC_CACHE = {}


def kernel(central_features, context_features, qw1, qw2, kw1, kw2,
           vw1, vw2, fw1, fw2, _trace=False, _return_results=False, **_kw):
    if "nc" not in _NC_CACHE:
        _NC_CACHE["nc"] = build_nc()
    nc = _NC_CACHE["nc"]

    bf16 = ml_dtypes.bfloat16
    weights = {"qw1": qw1, "qw2": qw2, "kw1": kw1, "kw2": kw2,
               "vw1": vw1, "vw2": np.asarray(vw2, np.float32) / float(SS),
               "fw1": fw1, "fw2": fw2}
    weights = {k: np.ascontiguousarray(np.asarray(v, np.float32).astype(bf16))
               for k, v in weights.items()}
    central_features = np.ascontiguousarray(
        np.asarray(central_features, np.float32).astype(bf16))
    context_features = np.ascontiguousarray(
        np.asarray(context_features, np.float32).astype(bf16))

    in_maps = []
    for r in range(M):
        m = {"central_sh": central_features[r * NS:(r + 1) * NS],
             "context_sh": context_features[r * TS:(r + 1) * TS]}
        m.update(weights)
        in_maps.append(m)

    res = run_bass_kernel_spmd(nc, in_maps, core_ids=list(range(M)),
                               trace=_trace)
    out = np.concatenate([res.results[r]["out_sh"] for r in range(M)], axis=0)
    if _return_results:
        return out, res
    return out


if __name__ == "__main__":
    rng = np.random.default_rng(0)
    f = lambda *s: rng.standard_normal(s, dtype=np.float32)
    ins = dict(central_features=f(N, C, S, S), context_features=f(T, C, S, S),
               qw1=f(C, H) * 0.02, qw2=f(H, D) * 0.02,
               kw1=f(C, H) * 0.02, kw2=f(H, D) * 0.02,
               vw1=f(C, H) * 0.02, vw2=f(H, D) * 0.02,
               fw1=f(D, H) * 0.02, fw2=f(H, C) * 0.02)
    out = kernel(**ins)
    print(out.shape, out.dtype, np.abs(out).max())
